# revision 6
# baseline (speedup 1.0000x reference)
"""Trainium2 Bass kernel for nn_CrossModalFusion.

Math: with seq_len=1 on both attention sides, softmax over the single key is
identically 1, so MHA collapses to  ctx = x_kv @ Wv.T @ Wo.T + (Wo @ bv + bo).
We fuse (Wv.T @ Wo.T) into one [d, d] weight on the host, so each modality is a
single [B,d]x[d,d] matmul, a residual add, a LayerNorm, plus the final
concat([img_out, txt_out, img_out*txt_out]).

Sharding: pure data parallel over the batch dim across 8 NeuronCores, weights
replicated, no collectives.

Device layout per core (Bs = 4096 rows):
  - x_nat  [Bs, d]  bf16  natural layout (residual input)
  - xT     [d, Bs]  bf16  host-transposed (matmul stationary operand: the PE
                          needs the contraction dim on partitions)
  - W      [d, d]   bf16  fused weight, [d_in, d_out]
  - out    [Bs, 3d] f32
"""

import os
import numpy as np
import ml_dtypes

B, D, NCORES = 32768, 1024, 8
BS = B // NCORES          # 4096 rows per core
PT = 128                  # partition tile (rows per b-tile)
NBT = BS // PT            # 32 b-tiles per core
KT = D // PT              # 8 k-tiles of the contraction
NH = 512                  # psum half width (one fp32 PSUM bank)
CHUNK = 1024              # b-columns per transposed-input chunk load
NCHUNK = BS // CHUNK
EPS = 1e-5
BF16 = ml_dtypes.bfloat16

_CACHE = {}


def _build_program(repeats, has_bias, has_affine):
    import concourse.bass as bass
    import concourse.tile as tile
    from concourse import bacc, mybir

    f32 = mybir.dt.float32
    bf16 = mybir.dt.bfloat16
    AF = mybir.ActivationFunctionType
    ALU = mybir.AluOpType

    nc = bacc.Bacc("TRN2", enable_partition_id=False)

    img_n = nc.declare_dram_parameter("img_n", [BS, D], bf16, isOutput=False)
    txt_n = nc.declare_dram_parameter("txt_n", [BS, D], bf16, isOutput=False)
    imgT = nc.declare_dram_parameter("imgT", [D, BS], bf16, isOutput=False)
    txtT = nc.declare_dram_parameter("txtT", [D, BS], bf16, isOutput=False)
    w_it = nc.declare_dram_parameter("w_it", [D, D], bf16, isOutput=False)
    w_ti = nc.declare_dram_parameter("w_ti", [D, D], bf16, isOutput=False)
    bias_d = affine_d = None
    if has_bias:
        # [2, D]: row 0 = it (img ctx) bias, row 1 = ti
        bias_d = nc.declare_dram_parameter("bias", [2, D], f32, isOutput=False)
    if has_affine:
        # [4, D]: g_img, b_img, g_txt, b_txt
        affine_d = nc.declare_dram_parameter("affine", [4, D], f32, isOutput=False)
    out_d = nc.declare_dram_parameter("out", [BS, 3 * D], f32, isOutput=True)

    with tile.TileContext(nc) as tc:
        with (
            tc.tile_pool(name="singles", bufs=1) as singles,
            tc.tile_pool(name="wpool", bufs=1) as wpool,
            tc.tile_pool(name="xtpool", bufs=2) as xtpool,
            tc.tile_pool(name="natpool", bufs=3) as natpool,
            tc.tile_pool(name="ypool", bufs=2) as ypool,
            tc.tile_pool(name="lnpool", bufs=3) as lnpool,
            tc.tile_pool(name="prodpool", bufs=2) as prodpool,
            tc.tile_pool(name="smalls", bufs=6) as smalls,
            tc.tile_pool(name="psum", bufs=2, space=bass.MemorySpace.PSUM) as psum,
        ):
            def body():
                eps_t = singles.tile([PT, 1], f32, tag="eps")
                nc.vector.memset(eps_t, EPS)

                w_sb = {}
                for mod, w_d in (("it", w_it), ("ti", w_ti)):
                    w = wpool.tile([PT, KT, D], bf16, tag=f"w_{mod}", name=f"w_{mod}")
                    nc.sync.dma_start(
                        out=w, in_=w_d.rearrange("(k p) n -> p k n", p=PT)
                    )
                    w_sb[mod] = w

                bias_bc = {}
                if has_bias:
                    for i, mod in enumerate(("it", "ti")):
                        t = singles.tile([PT, D], f32, tag=f"bias_{mod}",
                                         name=f"bias_{mod}")
                        src = bias_d[i : i + 1, :]
                        src = bass.AP(tensor=src.tensor, offset=src.offset,
                                      ap=[[0, PT], [1, D]])
                        nc.sync.dma_start(out=t, in_=src)
                        bias_bc[mod] = t
                aff_bc = {}
                if has_affine:
                    for i, nm in enumerate(("g_img", "b_img", "g_txt", "b_txt")):
                        t = singles.tile([PT, D], f32, tag=f"aff_{nm}", name=nm)
                        src = affine_d[i : i + 1, :]
                        src = bass.AP(tensor=src.tensor, offset=src.offset,
                                      ap=[[0, PT], [1, D]])
                        nc.sync.dma_start(out=t, in_=src)
                        aff_bc[nm] = t

                for c in range(NCHUNK):
                    xt_sb = {}
                    for mod, xT_d in (("it", txtT), ("ti", imgT)):
                        # "it" produces img_ctx from txt; "ti" the reverse
                        xt = xtpool.tile([PT, KT, CHUNK], bf16, tag=f"xt_{mod}",
                                         name=f"xt_{mod}")
                        nc.sync.dma_start(
                            out=xt,
                            in_=xT_d[:, c * CHUNK : (c + 1) * CHUNK].rearrange(
                                "(k p) b -> p k b", p=PT
                            ),
                        )
                        xt_sb[mod] = xt

                    for bb in range(CHUNK // PT):
                        b0 = c * CHUNK + bb * PT
                        rows = slice(b0, b0 + PT)
                        ln_sb = {}
                        for mod, x_nat_d, gb in (
                            ("it", img_n, ("g_img", "b_img")),
                            ("ti", txt_n, ("g_txt", "b_txt")),
                        ):
                            nat = natpool.tile([PT, D], bf16, tag=f"nat_{mod}",
                                               name=f"nat_{mod}")
                            nc.sync.dma_start(out=nat, in_=x_nat_d[rows, :])

                            ps = psum.tile([PT, D], f32, tag=f"ps_{mod}",
                                           name=f"ps_{mod}")
                            xt = xt_sb[mod]
                            for k in range(KT):
                                lhsT = xt[:, k, bb * PT : (bb + 1) * PT]
                                for h in range(2):
                                    nc.tensor.matmul(
                                        ps[:, h * NH : (h + 1) * NH],
                                        lhsT,
                                        w_sb[mod][:, k, h * NH : (h + 1) * NH],
                                        start=(k == 0),
                                        stop=(k == KT - 1),
                                    )

                            y = ypool.tile([PT, D], f32, tag=f"y_{mod}",
                                           name=f"y_{mod}")
                            nc.vector.tensor_add(y, ps, nat)
                            if has_bias:
                                nc.gpsimd.tensor_add(y, y, bias_bc[mod])

                            stats = smalls.tile([PT, 2, 6], f32, tag=f"st_{mod}",
                                                name=f"st_{mod}")
                            nc.vector.bn_stats(stats[:, 0, :], y[:, 0:NH])
                            nc.vector.bn_stats(stats[:, 1, :], y[:, NH:D])
                            mv = smalls.tile([PT, 2], f32, tag=f"mv_{mod}",
                                             name=f"mv_{mod}")
                            nc.vector.bn_aggr(mv, stats)

                            std = smalls.tile([PT, 1], f32, tag=f"sd_{mod}",
                                              name=f"sd_{mod}")
                            nc.scalar.activation(
                                std, mv[:, 1:2], func=AF.Sqrt, bias=eps_t, scale=1.0
                            )
                            rstd = smalls.tile([PT, 1], f32, tag=f"rs_{mod}",
                                               name=f"rs_{mod}")
                            nc.vector.reciprocal(rstd, std)

                            ln = lnpool.tile([PT, D], f32, tag=f"ln_{mod}",
                                             name=f"ln_{mod}")
                            # (y - mu) * rstd on POOL (1-input: line rate there)
                            nc.gpsimd.tensor_scalar(
                                ln, y, mv[:, 0:1], rstd,
                                op0=ALU.subtract, op1=ALU.mult,
                            )
                            if has_affine:
                                nc.gpsimd.tensor_mul(ln, ln, aff_bc[gb[0]])
                                nc.gpsimd.tensor_add(ln, ln, aff_bc[gb[1]])
                            ln_sb[mod] = ln

                            sect = 0 if mod == "it" else D
                            nc.sync.dma_start(
                                out=out_d[rows, sect : sect + D], in_=ln
                            )

                        prod = prodpool.tile([PT, D], f32, tag="prod", name="prod")
                        nc.gpsimd.tensor_mul(prod, ln_sb["it"], ln_sb["ti"])
                        nc.sync.dma_start(out=out_d[rows, 2 * D : 3 * D], in_=prod)

            if repeats == 1:
                body()
            else:
                with tc.For_i(0, repeats, 1):
                    body()

    nc.finalize()
    return nc


def _get_exec(repeats=1, has_bias=False, has_affine=False):
    key = (repeats, has_bias, has_affine)
    if key in _CACHE:
        return _CACHE[key]

    import jax
    from jax.experimental.shard_map import shard_map
    from jax.sharding import Mesh, PartitionSpec
    from concourse import mybir
    from concourse.bass2jax import (
        _bass_exec_p,
        install_neuronx_cc_hook,
        partition_id_tensor,
    )

    install_neuronx_cc_hook()
    nc = _build_program(repeats, has_bias, has_affine)

    partition_name = nc.partition_id_tensor.name if nc.partition_id_tensor else None
    in_names, out_names, out_avals = [], [], []
    for alloc in nc.m.functions[0].allocations:
        if not isinstance(alloc, mybir.MemoryLocationSet):
            continue
        name = alloc.memorylocations[0].name
        if alloc.kind == "ExternalInput":
            if name != partition_name:
                in_names.append(name)
        elif alloc.kind == "ExternalOutput":
            out_names.append(name)
            out_avals.append(
                jax.core.ShapedArray(tuple(alloc.tensor_shape), mybir.dt.np(alloc.dtype))
            )
    n_params = len(in_names)
    all_in_names = list(in_names) + out_names
    if partition_name is not None:
        all_in_names.append(partition_name)
    all_in_names = tuple(all_in_names)

    def _body(*args):
        operands = list(args)
        if partition_name is not None:
            operands.append(partition_id_tensor())
        return tuple(
            _bass_exec_p.bind(
                *operands,
                out_avals=tuple(out_avals),
                in_names=all_in_names,
                out_names=tuple(out_names),
                lowering_input_output_aliases=(),
                sim_require_finite=True,
                sim_require_nnan=True,
                nc=nc,
            )
        )

    devices = jax.devices()[:NCORES]
    assert len(devices) == NCORES, f"need {NCORES} devices, got {len(devices)}"
    mesh = Mesh(np.asarray(devices), ("core",))
    nspecs = n_params + len(out_names)
    fn = jax.jit(
        shard_map(
            _body,
            mesh=mesh,
            in_specs=(PartitionSpec("core"),) * nspecs,
            out_specs=(PartitionSpec("core"),) * len(out_names),
            check_rep=False,
        ),
        keep_unused=True,
    )
    entry = (fn, in_names, out_names, out_avals, mesh)
    _CACHE[key] = entry
    return entry


def _prep_inputs(inputs):
    """Host-side prep: fuse weights, cast, transpose. Returns (global input
    arrays dict keyed by dram param name, has_bias, has_affine)."""
    img = np.asarray(inputs["img"], np.float32)
    txt = np.asarray(inputs["txt"], np.float32)

    glob = {}
    has_bias = False
    bias_rows = []
    w_glob = {}
    for mod, wi, bi, wo, bo in (
        ("it", "Wi_it", "bi_it", "Wo_it", "bo_it"),
        ("ti", "Wi_ti", "bi_ti", "Wo_ti", "bo_ti"),
    ):
        Wi = np.asarray(inputs[wi], np.float32)
        Wo = np.asarray(inputs[wo], np.float32)
        bi = np.asarray(inputs[bi], np.float32)
        bo = np.asarray(inputs[bo], np.float32)
        Wv = Wi[2 * D : 3 * D]               # v = x_kv @ Wv.T + bv
        Wf = (Wv.T @ Wo.T).astype(BF16)      # ctx = x_kv @ Wf, [d_in, d_out]
        bf = Wo @ bi[2 * D : 3 * D] + bo
        w_glob[mod] = Wf
        bias_rows.append(bf)
        if np.any(bf != 0.0):
            has_bias = True

    aff = [np.asarray(inputs[k], np.float32)
           for k in ("g_img", "b_img", "g_txt", "b_txt")]
    has_affine = bool(
        np.any(aff[0] != 1.0) or np.any(aff[1] != 0.0)
        or np.any(aff[2] != 1.0) or np.any(aff[3] != 0.0)
    )

    img16 = img.astype(BF16)
    txt16 = txt.astype(BF16)
    glob["img_n"] = img16
    glob["txt_n"] = txt16
    # Per-core transposed shards, concatenated along axis 0 for shard_map.
    glob["imgT"] = np.ascontiguousarray(
        img16.reshape(NCORES, BS, D).transpose(0, 2, 1)
    ).reshape(NCORES * D, BS)
    glob["txtT"] = np.ascontiguousarray(
        txt16.reshape(NCORES, BS, D).transpose(0, 2, 1)
    ).reshape(NCORES * D, BS)
    glob["w_it"] = np.broadcast_to(w_glob["it"], (NCORES, D, D)).reshape(NCORES * D, D).copy()
    glob["w_ti"] = np.broadcast_to(w_glob["ti"], (NCORES, D, D)).reshape(NCORES * D, D).copy()
    if has_bias:
        b = np.stack(bias_rows).astype(np.float32)  # [2, D]
        glob["bias"] = np.broadcast_to(b, (NCORES, 2, D)).reshape(NCORES * 2, D).copy()
    if has_affine:
        a = np.stack(aff).astype(np.float32)  # [4, D]
        glob["affine"] = np.broadcast_to(a, (NCORES, 4, D)).reshape(NCORES * 4, D).copy()
    return glob, has_bias, has_affine


def kernel(**inputs):
    glob, has_bias, has_affine = _prep_inputs(inputs)
    fn, in_names, out_names, out_avals, mesh = _get_exec(1, has_bias, has_affine)
    args = [glob[n] for n in in_names]
    zeros = [
        np.zeros((NCORES * av.shape[0], *av.shape[1:]), av.dtype) for av in out_avals
    ]
    outs = fn(*args, *zeros)
    return np.asarray(outs[0])


if __name__ == "__main__":
    rng = np.random.default_rng(0)
    fake = {
        "img": rng.standard_normal((B, D), np.float32),
        "txt": rng.standard_normal((B, D), np.float32),
        "Wi_it": rng.standard_normal((3 * D, D), np.float32) / 32,
        "bi_it": np.zeros(3 * D, np.float32),
        "Wo_it": rng.standard_normal((D, D), np.float32) / 32,
        "bo_it": np.zeros(D, np.float32),
        "Wi_ti": rng.standard_normal((3 * D, D), np.float32) / 32,
        "bi_ti": np.zeros(3 * D, np.float32),
        "Wo_ti": rng.standard_normal((D, D), np.float32) / 32,
        "bo_ti": np.zeros(D, np.float32),
        "g_img": np.ones(D, np.float32),
        "b_img": np.zeros(D, np.float32),
        "g_txt": np.ones(D, np.float32),
        "b_txt": np.zeros(D, np.float32),
    }
    out = kernel(**fake)
    print(out.shape, out.dtype)
